# revision 1
# baseline (speedup 1.0000x reference)
"""DeepseekV3 mini MoE MLP on 8 TRN2 NeuronCores.

Strategy: expert-parallel. The router (tiny: 0.1% of FLOPs) is computed
with jax ops that mirror the reference bit-for-bit; tokens are then
dispatched on the host to per-expert batches (the "all-to-all"), one
expert per NeuronCore. Each core runs a fused gate/up/silu/mul/down
kernel over its routed tokens in f32r (FP22 single-pass matmul — full
TensorE rate, fp32 storage). The combine (scatter-add weighted by the
top-k routing weights) happens on the host.

Layouts are feature-major ([dim, tokens]) so every matmul contracts over
the SBUF partition dim with no transposes anywhere on device. Weights
are passed pre-chunked ([HT, P, DT, P]) so each output-column block's
weights arrive in one fully-contiguous DMA, letting the first matmuls
start ~10us into the kernel instead of waiting for the full 12.6MB.
"""

import numpy as np

import concourse.bass as bass
import concourse.mybir as mybir
import concourse.tile as tile
from concourse import bacc
from concourse.bass_utils import run_bass_kernel_spmd

DIM = 1024
HIDDEN = 1024
NUM_EXPERTS = 8
TOP_K = 2
P = 128
TT = 512  # main token tile (PSUM bank = 512 fp32)
DT = DIM // P  # 8 d-tiles
HT = HIDDEN // P  # 8 h-tiles

F32 = mybir.dt.float32
F32R = mybir.dt.float32r

_program_cache: dict[tuple, object] = {}
LAST_RESULT = None


def _build_expert_program(tiles: tuple):
    """One-expert FFN: yt = ((silu(x@wg.T) * (x@wu.T)) @ wd.T).T over C tokens.

    DRAM params (per core):
      xt [DIM, C]            tokens, transposed (d-major)
      wg/wu [HT, P, DT, P]   gate/up proj, chunked: [h-blk, d-in, d-blk, h-in]
      wd [HT, P, HT, P]      down proj, chunked: [dout-blk, h-in, h-blk, dout-in]
      yt [DIM, C]            output, transposed
    """
    C = sum(tiles)
    nc = bacc.Bacc(None, target_bir_lowering=False, debug=False)
    xt = nc.declare_dram_parameter("xt", [DIM, C], F32R, isOutput=False)
    wg = nc.declare_dram_parameter("wg", [HT, P, DT, P], F32R, isOutput=False)
    wu = nc.declare_dram_parameter("wu", [HT, P, DT, P], F32R, isOutput=False)
    wd = nc.declare_dram_parameter("wd", [HT, P, HT, P], F32R, isOutput=False)
    yt = nc.declare_dram_parameter("yt", [DIM, C], F32, isOutput=True)

    with tile.TileContext(nc) as tc:
        with (
            tc.tile_pool(name="wpool", bufs=1) as wpool,
            tc.tile_pool(name="xpool", bufs=2) as xpool,
            tc.tile_pool(name="hpool", bufs=2) as hpool,
            tc.tile_pool(name="apool", bufs=3) as apool,
            tc.tile_pool(name="ypool", bufs=3) as ypool,
            tc.tile_pool(name="pg", bufs=2, space="PSUM") as pgpool,
            tc.tile_pool(name="pu", bufs=2, space="PSUM") as pupool,
            tc.tile_pool(name="py", bufs=2, space="PSUM") as pypool,
        ):
            # Weight chunk tiles: one per output-column block, so matmuls for
            # block k only depend on chunk k's DMA (fast pipeline ramp).
            wg_c, wu_c, wd_c = [], [], []
            for k in range(HT):
                wg_c.append(wpool.tile([P, DT * P], F32R, name=f"wg{k}", tag=f"wg{k}"))
                wu_c.append(wpool.tile([P, DT * P], F32R, name=f"wu{k}", tag=f"wu{k}"))
            for k in range(HT):
                wd_c.append(wpool.tile([P, HT * P], F32R, name=f"wd{k}", tag=f"wd{k}"))

            first = True
            off = 0
            for t, tt in enumerate(tiles):
                ts = bass.ds(off, tt)
                off += tt
                x_sb = xpool.tile([P, DT * TT], F32R, tag="x")
                nc.sync.dma_start(
                    out=x_sb[:, :].rearrange("p (a t) -> p a t", a=DT)[:, :, :tt],
                    in_=xt.ap()[:, ts].rearrange("(a p) t -> p a t", p=P),
                )
                if first:
                    # Weight DMAs issued after the first x tile: each chunk is
                    # contiguous in DRAM; block-k matmuls start as soon as
                    # chunk k lands.
                    for k in range(HT):
                        nc.sync.dma_start(out=wg_c[k][:, :], in_=wg.ap()[k])
                        nc.sync.dma_start(out=wu_c[k][:, :], in_=wu.ap()[k])
                    for k in range(HT):
                        nc.sync.dma_start(out=wd_c[k][:, :], in_=wd.ap()[k])
                    first = False

                h_sb = hpool.tile([P, HT * TT], F32R, tag="h")
                for h in range(HT):
                    pg = pgpool.tile([P, tt], F32, tag="pg")
                    pu = pupool.tile([P, tt], F32, tag="pu")
                    for a in range(DT):
                        nc.tensor.matmul(
                            pg[:, :],
                            wg_c[h][:, a * P : (a + 1) * P],
                            x_sb[:, a * TT : a * TT + tt],
                            start=(a == 0),
                            stop=(a == DT - 1),
                        )
                    for a in range(DT):
                        nc.tensor.matmul(
                            pu[:, :],
                            wu_c[h][:, a * P : (a + 1) * P],
                            x_sb[:, a * TT : a * TT + tt],
                            start=(a == 0),
                            stop=(a == DT - 1),
                        )
                    act_sb = apool.tile([P, TT], F32, tag="act")
                    nc.scalar.activation(
                        act_sb[:, :tt], pg[:, :], mybir.ActivationFunctionType.Sigmoid
                    )
                    sil_sb = apool.tile([P, TT], F32, tag="sil")
                    nc.vector.tensor_tensor(
                        sil_sb[:, :tt], act_sb[:, :tt], pg[:, :], mybir.AluOpType.mult
                    )
                    nc.vector.tensor_tensor(
                        h_sb[:, h * TT : h * TT + tt],
                        sil_sb[:, :tt],
                        pu[:, :],
                        mybir.AluOpType.mult,
                    )

                for do in range(HT):
                    py = pypool.tile([P, tt], F32, tag="py")
                    for a in range(HT):
                        nc.tensor.matmul(
                            py[:, :],
                            wd_c[do][:, a * P : (a + 1) * P],
                            h_sb[:, a * TT : a * TT + tt],
                            start=(a == 0),
                            stop=(a == HT - 1),
                        )
                    y_sb = ypool.tile([P, TT], F32, tag="y")
                    nc.scalar.copy(y_sb[:, :tt], py[:, :])
                    nc.sync.dma_start(
                        out=yt.ap()[do * P : (do + 1) * P, ts], in_=y_sb[:, :tt]
                    )
    nc.compile()
    return nc


def _tiles_for(max_cnt: int) -> tuple:
    """Token tiles covering max_cnt: full 512s plus one final tile (>=256 for
    full-rate f32r, multiple of 64)."""
    full, rem = divmod(max_cnt, TT)
    if rem == 0:
        return (TT,) * max(full, 1)
    rem = max(256, ((rem + 63) // 64) * 64)
    if rem == TT:
        return (TT,) * (full + 1)
    return (TT,) * full + (rem,)


def _get_program(tiles: tuple):
    if tiles not in _program_cache:
        _program_cache[tiles] = _build_expert_program(tiles)
    return _program_cache[tiles]


def _chunk_w(wt: np.ndarray) -> np.ndarray:
    """[K, M] weight (K contracted) -> chunk layout [m_blk, k_in, k_blk, m_in],
    contiguous per m_blk."""
    K, M = wt.shape
    # wt[k, m] with k = kb*P + kp, m = mb*P + mp  ->  out[mb, kp, kb, mp]
    return np.ascontiguousarray(wt.reshape(K // P, P, M // P, P).transpose(2, 1, 0, 3))


def _route(flat: np.ndarray, gate_w: np.ndarray):
    """Mirror the reference router bit-for-bit (jax ops, same backend)."""
    try:
        import jax
        import jax.numpy as jnp

        logits = jnp.asarray(flat) @ jnp.asarray(gate_w).T
        scores = jax.nn.sigmoid(logits)
        top_val, top_idx = jax.lax.top_k(scores, TOP_K)
        top_val = top_val / (top_val.sum(-1, keepdims=True) + 1e-9)
        return np.asarray(top_val), np.asarray(top_idx)
    except Exception:
        # numpy fallback: identical selection semantics (stable descending)
        logits = flat @ gate_w.T
        scores = 1.0 / (1.0 + np.exp(-logits))
        order = np.argsort(-scores, axis=-1, kind="stable")
        top_idx = order[:, :TOP_K].astype(np.int32)
        top_val = np.take_along_axis(scores, top_idx, axis=-1)
        top_val = top_val / (top_val.sum(-1, keepdims=True) + 1e-9)
        return top_val.astype(np.float32), top_idx


def kernel(x, gate_w, gate_proj, up_proj, down_proj):
    x = np.asarray(x)
    bsz, seqlen, dim = x.shape
    flat = np.ascontiguousarray(x.reshape(-1, dim), dtype=np.float32)
    T = flat.shape[0]
    gate_w = np.asarray(gate_w, dtype=np.float32)
    gate_proj = np.asarray(gate_proj, dtype=np.float32)
    up_proj = np.asarray(up_proj, dtype=np.float32)
    down_proj = np.asarray(down_proj, dtype=np.float32)

    top_val, top_idx = _route(flat, gate_w)

    idx_list = []
    cw_list = []
    for e in range(NUM_EXPERTS):
        mask = top_idx == e  # [T, K]
        tok = np.nonzero(mask.any(axis=1))[0]
        w = (top_val * mask).sum(axis=1)[tok].astype(np.float32)
        idx_list.append(tok)
        cw_list.append(w)

    max_cnt = max(len(i) for i in idx_list)
    tiles = _tiles_for(max_cnt)
    C = sum(tiles)
    nc = _get_program(tiles)

    in_maps = []
    for e in range(NUM_EXPERTS):
        tok = idx_list[e]
        cnt = len(tok)
        xt = np.zeros((DIM, C), dtype=np.float32)
        xt[:, :cnt] = flat[tok].T
        in_maps.append(
            {
                "xt": xt,
                "wg": _chunk_w(gate_proj[e].T),
                "wu": _chunk_w(up_proj[e].T),
                "wd": _chunk_w(down_proj[e].T),
            }
        )

    res = run_bass_kernel_spmd(nc, in_maps, core_ids=list(range(NUM_EXPERTS)))
    global LAST_RESULT
    LAST_RESULT = res

    out = np.zeros((T, DIM), dtype=np.float32)
    for e in range(NUM_EXPERTS):
        tok = idx_list[e]
        cnt = len(tok)
        if cnt:
            out[tok] += (res.results[e]["yt"][:, :cnt] * cw_list[e][None, :]).T
    return out.reshape(bsz, seqlen, dim)



# revision 2
# speedup vs baseline: 1.0688x; 1.0688x over previous
"""DeepseekV3 mini MoE MLP on 8 TRN2 NeuronCores.

Strategy: expert-parallel. The router (tiny: 0.1% of FLOPs) is computed
with jax ops that mirror the reference bit-for-bit; tokens are then
dispatched on the host to per-expert batches (the "all-to-all"), one
expert per NeuronCore. Each core runs a fused gate/up/silu/mul/down
kernel over its routed tokens. The combine (scatter-add weighted by the
top-k routing weights) happens on the host.

All tensor data is fp16 (10-bit mantissa; rel-err ~5e-4 end to end):
matmuls run at the same 1 elem/cycle TensorE rate as f32r, but
LDWEIGHTS gets FastWeightLoad (2x) so the MM stream runs at the
~216ns/512-col floor, and DMA bytes halve. PSUM accumulation stays
fp32.

Layouts are feature-major ([dim, tokens]) so every matmul contracts
over the SBUF partition dim with no transposes anywhere on device.
x arrives in per-128-row chunk tiles so the first matmul only waits
for 1/8th of the first token tile; weights are passed pre-chunked
([HT, P, DT, P]) so each output-column block's weights arrive in one
fully-contiguous DMA. A short burst of zero-weight warmup matmuls runs
during the initial DMA fill to lift the PE out of its cold (1.2 GHz)
HAM state before real work starts.
"""

import numpy as np

import concourse.bass as bass
import concourse.mybir as mybir
import concourse.tile as tile
from concourse import bacc
from concourse.bass_utils import run_bass_kernel_spmd

DIM = 1024
HIDDEN = 1024
NUM_EXPERTS = 8
TOP_K = 2
P = 128
TT = 512  # main token tile (PSUM bank = 512 fp32)
DT = DIM // P  # 8 d-tiles
HT = HIDDEN // P  # 8 h-tiles

F32 = mybir.dt.float32
F16 = mybir.dt.float16

_program_cache: dict[tuple, object] = {}
LAST_RESULT = None


def _build_expert_program(tiles: tuple):
    """One-expert FFN: yt = ((silu(x@wg.T) * (x@wu.T)) @ wd.T).T over C tokens.

    DRAM params (per core), all fp16:
      xt [DIM, C]            tokens, transposed (d-major)
      wg/wu [HT, P, DT, P]   gate/up proj, chunked: [h-blk, d-in, d-blk, h-in]
      wd [HT, P, HT, P]      down proj, chunked: [dout-blk, h-in, h-blk, dout-in]
      yt [DIM, C]            output, transposed
    """
    C = sum(tiles)
    nc = bacc.Bacc(None, target_bir_lowering=False, debug=False)
    xt = nc.declare_dram_parameter("xt", [DIM, C], F16, isOutput=False)
    wg = nc.declare_dram_parameter("wg", [HT, P, DT, P], F16, isOutput=False)
    wu = nc.declare_dram_parameter("wu", [HT, P, DT, P], F16, isOutput=False)
    wd = nc.declare_dram_parameter("wd", [HT, P, HT, P], F16, isOutput=False)
    yt = nc.declare_dram_parameter("yt", [DIM, C], F16, isOutput=True)

    with tile.TileContext(nc) as tc:
        with (
            tc.tile_pool(name="wpool", bufs=1) as wpool,
            tc.tile_pool(name="xpool", bufs=2) as xpool,
            tc.tile_pool(name="hpool", bufs=2) as hpool,
            tc.tile_pool(name="apool", bufs=3) as apool,
            tc.tile_pool(name="ypool", bufs=3) as ypool,
            tc.tile_pool(name="pg", bufs=2, space="PSUM") as pgpool,
            tc.tile_pool(name="pu", bufs=2, space="PSUM") as pupool,
            tc.tile_pool(name="py", bufs=2, space="PSUM") as pypool,
            tc.tile_pool(name="pw", bufs=1, space="PSUM") as pwpool,
        ):
            # PE warmup: zero-weight matmuls issued first in program order.
            # They have no DMA deps, so they run during the initial x/weight
            # fill and bring the PE HAM out of its cold half-rate state.
            zt = wpool.tile([P, TT], F16, name="zwarm", tag="zwarm")
            nc.vector.memset(zt[:, :], 0.0)
            pwarm = pwpool.tile([P, TT], F32, tag="pwarm")
            for _ in range(14):
                nc.tensor.matmul(
                    pwarm[:, :], zt[:, :P], zt[:, :], start=True, stop=True
                )

            # Weight chunk tiles: one per output-column block, so matmuls for
            # block k only depend on chunk k's DMA (fast pipeline ramp).
            wg_c, wu_c, wd_c = [], [], []
            for k in range(HT):
                wg_c.append(wpool.tile([P, DT * P], F16, name=f"wg{k}", tag=f"wg{k}"))
                wu_c.append(wpool.tile([P, DT * P], F16, name=f"wu{k}", tag=f"wu{k}"))
            for k in range(HT):
                wd_c.append(wpool.tile([P, HT * P], F16, name=f"wd{k}", tag=f"wd{k}"))

            first = True
            off = 0
            for t, tt in enumerate(tiles):
                ts = bass.ds(off, tt)
                off += tt
                # x in DT per-chunk tiles: first MMs wait on 1 chunk, not 8.
                x_sb = []
                for a in range(DT):
                    xa = xpool.tile([P, TT], F16, tag=f"x{a}")
                    x_sb.append(xa)
                    nc.sync.dma_start(
                        out=xa[:, :tt], in_=xt.ap()[a * P : (a + 1) * P, ts]
                    )
                    if first and a == 0:
                        # First-block weights right behind the first x chunk.
                        nc.sync.dma_start(out=wg_c[0][:, :], in_=wg.ap()[0])
                        nc.sync.dma_start(out=wu_c[0][:, :], in_=wu.ap()[0])
                if first:
                    for k in range(1, HT):
                        nc.sync.dma_start(out=wg_c[k][:, :], in_=wg.ap()[k])
                        nc.sync.dma_start(out=wu_c[k][:, :], in_=wu.ap()[k])
                    for k in range(HT):
                        nc.sync.dma_start(out=wd_c[k][:, :], in_=wd.ap()[k])
                    first = False

                h_sb = hpool.tile([P, HT * TT], F16, tag="h")
                for h in range(HT):
                    pg = pgpool.tile([P, tt], F32, tag="pg")
                    pu = pupool.tile([P, tt], F32, tag="pu")
                    for a in range(DT):
                        nc.tensor.matmul(
                            pg[:, :],
                            wg_c[h][:, a * P : (a + 1) * P],
                            x_sb[a][:, :tt],
                            start=(a == 0),
                            stop=(a == DT - 1),
                        )
                    for a in range(DT):
                        nc.tensor.matmul(
                            pu[:, :],
                            wu_c[h][:, a * P : (a + 1) * P],
                            x_sb[a][:, :tt],
                            start=(a == 0),
                            stop=(a == DT - 1),
                        )
                    act_sb = apool.tile([P, TT], F32, tag="act")
                    nc.scalar.activation(
                        act_sb[:, :tt], pg[:, :], mybir.ActivationFunctionType.Sigmoid
                    )
                    sil_sb = apool.tile([P, TT], F32, tag="sil")
                    nc.vector.tensor_tensor(
                        sil_sb[:, :tt], act_sb[:, :tt], pg[:, :], mybir.AluOpType.mult
                    )
                    nc.vector.tensor_tensor(
                        h_sb[:, h * TT : h * TT + tt],
                        sil_sb[:, :tt],
                        pu[:, :],
                        mybir.AluOpType.mult,
                    )

                for do in range(HT):
                    py = pypool.tile([P, tt], F32, tag="py")
                    for a in range(HT):
                        nc.tensor.matmul(
                            py[:, :],
                            wd_c[do][:, a * P : (a + 1) * P],
                            h_sb[:, a * TT : a * TT + tt],
                            start=(a == 0),
                            stop=(a == HT - 1),
                        )
                    y_sb = ypool.tile([P, TT], F16, tag="y")
                    nc.scalar.copy(y_sb[:, :tt], py[:, :])
                    nc.sync.dma_start(
                        out=yt.ap()[do * P : (do + 1) * P, ts], in_=y_sb[:, :tt]
                    )
    nc.compile()
    return nc


def _tiles_for(max_cnt: int) -> tuple:
    """Token tiles covering max_cnt: full 512s plus one final tile (fp16 runs
    full-rate at any N; keep a multiple of 8 for DMA niceness)."""
    full, rem = divmod(max_cnt, TT)
    if rem == 0:
        return (TT,) * max(full, 1)
    rem = max(64, ((rem + 7) // 8) * 8)
    if rem == TT:
        return (TT,) * (full + 1)
    return (TT,) * full + (rem,)


def _get_program(tiles: tuple):
    if tiles not in _program_cache:
        _program_cache[tiles] = _build_expert_program(tiles)
    return _program_cache[tiles]


def _chunk_w(wt: np.ndarray) -> np.ndarray:
    """[K, M] weight (K contracted) -> chunk layout [m_blk, k_in, k_blk, m_in],
    contiguous per m_blk."""
    K, M = wt.shape
    # wt[k, m] with k = kb*P + kp, m = mb*P + mp  ->  out[mb, kp, kb, mp]
    return np.ascontiguousarray(wt.reshape(K // P, P, M // P, P).transpose(2, 1, 0, 3))


def _route(flat: np.ndarray, gate_w: np.ndarray):
    """Mirror the reference router bit-for-bit (jax ops, same backend)."""
    try:
        import jax
        import jax.numpy as jnp

        logits = jnp.asarray(flat) @ jnp.asarray(gate_w).T
        scores = jax.nn.sigmoid(logits)
        top_val, top_idx = jax.lax.top_k(scores, TOP_K)
        top_val = top_val / (top_val.sum(-1, keepdims=True) + 1e-9)
        return np.asarray(top_val), np.asarray(top_idx)
    except Exception:
        # numpy fallback: identical selection semantics (stable descending)
        logits = flat @ gate_w.T
        scores = 1.0 / (1.0 + np.exp(-logits))
        order = np.argsort(-scores, axis=-1, kind="stable")
        top_idx = order[:, :TOP_K].astype(np.int32)
        top_val = np.take_along_axis(scores, top_idx, axis=-1)
        top_val = top_val / (top_val.sum(-1, keepdims=True) + 1e-9)
        return top_val.astype(np.float32), top_idx


def kernel(x, gate_w, gate_proj, up_proj, down_proj):
    x = np.asarray(x)
    bsz, seqlen, dim = x.shape
    flat = np.ascontiguousarray(x.reshape(-1, dim), dtype=np.float32)
    T = flat.shape[0]
    gate_w = np.asarray(gate_w, dtype=np.float32)
    gate_proj = np.asarray(gate_proj, dtype=np.float32)
    up_proj = np.asarray(up_proj, dtype=np.float32)
    down_proj = np.asarray(down_proj, dtype=np.float32)

    top_val, top_idx = _route(flat, gate_w)

    idx_list = []
    cw_list = []
    for e in range(NUM_EXPERTS):
        mask = top_idx == e  # [T, K]
        tok = np.nonzero(mask.any(axis=1))[0]
        w = (top_val * mask).sum(axis=1)[tok].astype(np.float32)
        idx_list.append(tok)
        cw_list.append(w)

    max_cnt = max(len(i) for i in idx_list)
    tiles = _tiles_for(max_cnt)
    C = sum(tiles)
    nc = _get_program(tiles)

    flat16 = flat.astype(np.float16)
    in_maps = []
    for e in range(NUM_EXPERTS):
        tok = idx_list[e]
        cnt = len(tok)
        xt = np.zeros((DIM, C), dtype=np.float16)
        xt[:, :cnt] = flat16[tok].T
        in_maps.append(
            {
                "xt": xt,
                "wg": _chunk_w(gate_proj[e].T).astype(np.float16),
                "wu": _chunk_w(up_proj[e].T).astype(np.float16),
                "wd": _chunk_w(down_proj[e].T).astype(np.float16),
            }
        )

    res = run_bass_kernel_spmd(nc, in_maps, core_ids=list(range(NUM_EXPERTS)))
    global LAST_RESULT
    LAST_RESULT = res

    out = np.zeros((T, DIM), dtype=np.float32)
    for e in range(NUM_EXPERTS):
        tok = idx_list[e]
        cnt = len(tok)
        if cnt:
            y = res.results[e]["yt"][:, :cnt].astype(np.float32)
            out[tok] += (y * cw_list[e][None, :]).T
    return out.reshape(bsz, seqlen, dim)


# revision 3
# speedup vs baseline: 1.0754x; 1.0061x over previous
"""DeepseekV3 mini MoE MLP on 8 TRN2 NeuronCores.

Strategy: expert-parallel with overflow rebalancing. The router (tiny:
0.1% of FLOPs) is computed with jax ops that mirror the reference
bit-for-bit; tokens are then dispatched on the host to per-expert
batches (the "all-to-all"). Core c runs expert c over up to CAP=8192
of its routed tokens (segment A); tokens beyond CAP on heavy experts
are shipped to other cores' fixed-size segment B (SB=128 tokens, with
that expert's weights as per-core data), so every core does the same
8320 token-slots instead of padding all cores to the heaviest expert.
The combine (scatter-add weighted by the top-k routing weights)
happens on the host.

All tensor data is fp16 (10-bit mantissa; rel-err ~5e-4 end to end):
matmuls run at the same 1 elem/cycle TensorE rate as f32r, but
LDWEIGHTS gets FastWeightLoad so the MM stream runs at the
~216ns/512-col floor, and DMA bytes halve. PSUM accumulation is fp32.

Layouts are feature-major ([dim, tokens]) so every matmul contracts
over the SBUF partition dim with no transposes anywhere on device.
x arrives in per-128-row chunk tiles so the first matmul only waits
for 1/8th of the first token tile; weights are passed pre-chunked
([HT, P, DT, P]) so each output-column block's weights arrive in one
fully-contiguous DMA. A short burst of zero-weight warmup matmuls runs
during the initial DMA fill to lift the PE out of its cold (1.2 GHz)
HAM state before real work starts.
"""

import numpy as np

import concourse.bass as bass
import concourse.mybir as mybir
import concourse.tile as tile
from concourse import bacc
from concourse.bass_utils import run_bass_kernel_spmd

DIM = 1024
HIDDEN = 1024
NUM_EXPERTS = 8
TOP_K = 2
P = 128
TT = 512  # main token tile (PSUM bank = 512 fp32)
DT = DIM // P  # 8 d-tiles
HT = HIDDEN // P  # 8 h-tiles
CAP = 8192  # segment-A token slots per core (= avg load, 16 tiles)
SB = 128  # segment-B token slots per core (overflow rebalancing)

F32 = mybir.dt.float32
F16 = mybir.dt.float16

_program_cache: dict[tuple, object] = {}
LAST_RESULT = None


def _build_program(tiles_a: tuple, sb: int):
    """Two-segment FFN: yt = ((silu(x@wg.T) * (x@wu.T)) @ wd.T).T.

    Segment A: sum(tiles_a) tokens with weight set A (the core's own
    expert). Segment B: sb tokens with weight set B (overflow from a
    heavy expert). All params fp16; chunked weight layout
    [m_blk, k_in, k_blk, m_in] so each output block's weights are one
    contiguous DMA.
    """
    CA = sum(tiles_a)
    nc = bacc.Bacc(None, target_bir_lowering=False, debug=False)
    xt = nc.declare_dram_parameter("xt", [DIM, CA], F16, isOutput=False)
    wg = nc.declare_dram_parameter("wg", [HT, P, DT, P], F16, isOutput=False)
    wu = nc.declare_dram_parameter("wu", [HT, P, DT, P], F16, isOutput=False)
    wd = nc.declare_dram_parameter("wd", [HT, P, HT, P], F16, isOutput=False)
    yt = nc.declare_dram_parameter("yt", [DIM, CA], F16, isOutput=True)
    if sb:
        xtb = nc.declare_dram_parameter("xtb", [DIM, sb], F16, isOutput=False)
        wgb = nc.declare_dram_parameter("wgb", [HT, P, DT, P], F16, isOutput=False)
        wub = nc.declare_dram_parameter("wub", [HT, P, DT, P], F16, isOutput=False)
        wdb = nc.declare_dram_parameter("wdb", [HT, P, HT, P], F16, isOutput=False)
        ytb = nc.declare_dram_parameter("ytb", [DIM, sb], F16, isOutput=True)

    with tile.TileContext(nc) as tc:
        with (
            tc.tile_pool(name="wpool", bufs=1) as wpool,
            tc.tile_pool(name="xpool", bufs=2) as xpool,
            tc.tile_pool(name="hpool", bufs=2) as hpool,
            tc.tile_pool(name="apool", bufs=4) as apool,
            tc.tile_pool(name="ypool", bufs=4) as ypool,
            tc.tile_pool(name="pg", bufs=2, space="PSUM") as pgpool,
            tc.tile_pool(name="pu", bufs=2, space="PSUM") as pupool,
            tc.tile_pool(name="py", bufs=3, space="PSUM") as pypool,
        ):
            # PE warmup: zero-weight matmuls issued first in program order.
            # No DMA deps, so they run during the initial x/weight fill and
            # lift the PE HAM out of its cold half-rate state (~3.4us).
            zt = wpool.tile([P, TT], F16, name="zwarm", tag="zwarm")
            nc.vector.memset(zt[:, :], 0.0)
            pwarm = pypool.tile([P, TT], F32, tag="py")
            for _ in range(8):
                nc.tensor.matmul(
                    pwarm[:, :], zt[:, :P], zt[:, :], start=True, stop=True
                )

            def mk_wtiles(pfx):
                wg_c, wu_c, wd_c = [], [], []
                for k in range(HT):
                    wg_c.append(
                        wpool.tile([P, DT * P], F16, name=f"{pfx}wg{k}", tag=f"{pfx}wg{k}")
                    )
                    wu_c.append(
                        wpool.tile([P, DT * P], F16, name=f"{pfx}wu{k}", tag=f"{pfx}wu{k}")
                    )
                for k in range(HT):
                    wd_c.append(
                        wpool.tile([P, HT * P], F16, name=f"{pfx}wd{k}", tag=f"{pfx}wd{k}")
                    )
                return wg_c, wu_c, wd_c

            wA = mk_wtiles("a")
            wB = mk_wtiles("b") if sb else None

            def emit_tile(tt, ts, xsrc, ydst, w3, tagpfx):
                wg_c, wu_c, wd_c = w3
                x_sb = []
                for a in range(DT):
                    xa = xpool.tile([P, TT], F16, tag=f"x{a}")
                    x_sb.append(xa)
                    nc.sync.dma_start(
                        out=xa[:, :tt], in_=xsrc.ap()[a * P : (a + 1) * P, ts]
                    )
                h_sb = hpool.tile([P, HT * TT], F16, tag="h")
                for h in range(HT):
                    pg = pgpool.tile([P, tt], F32, tag="pg")
                    pu = pupool.tile([P, tt], F32, tag="pu")
                    for a in range(DT):
                        nc.tensor.matmul(
                            pg[:, :],
                            wg_c[h][:, a * P : (a + 1) * P],
                            x_sb[a][:, :tt],
                            start=(a == 0),
                            stop=(a == DT - 1),
                        )
                    for a in range(DT):
                        nc.tensor.matmul(
                            pu[:, :],
                            wu_c[h][:, a * P : (a + 1) * P],
                            x_sb[a][:, :tt],
                            start=(a == 0),
                            stop=(a == DT - 1),
                        )
                    act_sb = apool.tile([P, TT], F32, tag="act")
                    nc.scalar.activation(
                        act_sb[:, :tt], pg[:, :], mybir.ActivationFunctionType.Sigmoid
                    )
                    sil_sb = apool.tile([P, TT], F32, tag="sil")
                    nc.vector.tensor_tensor(
                        sil_sb[:, :tt], act_sb[:, :tt], pg[:, :], mybir.AluOpType.mult
                    )
                    nc.vector.tensor_tensor(
                        h_sb[:, h * TT : h * TT + tt],
                        sil_sb[:, :tt],
                        pu[:, :],
                        mybir.AluOpType.mult,
                    )
                for do in range(HT):
                    py = pypool.tile([P, tt], F32, tag="py")
                    for a in range(HT):
                        nc.tensor.matmul(
                            py[:, :],
                            wd_c[do][:, a * P : (a + 1) * P],
                            h_sb[:, a * TT : a * TT + tt],
                            start=(a == 0),
                            stop=(a == HT - 1),
                        )
                    y_sb = ypool.tile([P, TT], F16, tag="y")
                    nc.scalar.copy(y_sb[:, :tt], py[:, :])
                    nc.gpsimd.dma_start(
                        out=ydst.ap()[do * P : (do + 1) * P, ts], in_=y_sb[:, :tt]
                    )

            first = True
            off = 0
            for t, tt in enumerate(tiles_a):
                ts = bass.ds(off, tt)
                off += tt
                if first:
                    # Weight DMAs (gpsimd queue, off the x-chunk sync queue):
                    # first-block gate/up chunks first so block-0 matmuls can
                    # start as soon as x chunk 0 lands.
                    nc.gpsimd.dma_start(out=wA[0][0][:, :], in_=wg.ap()[0])
                    nc.gpsimd.dma_start(out=wA[1][0][:, :], in_=wu.ap()[0])
                    for k in range(1, HT):
                        nc.gpsimd.dma_start(out=wA[0][k][:, :], in_=wg.ap()[k])
                        nc.gpsimd.dma_start(out=wA[1][k][:, :], in_=wu.ap()[k])
                    for k in range(HT):
                        nc.gpsimd.dma_start(out=wA[2][k][:, :], in_=wd.ap()[k])
                    if sb:
                        for k in range(HT):
                            nc.gpsimd.dma_start(out=wB[0][k][:, :], in_=wgb.ap()[k])
                            nc.gpsimd.dma_start(out=wB[1][k][:, :], in_=wub.ap()[k])
                        for k in range(HT):
                            nc.gpsimd.dma_start(out=wB[2][k][:, :], in_=wdb.ap()[k])
                    first = False
                emit_tile(tt, ts, xt, yt, wA, "a")
            if sb:
                emit_tile(sb, bass.ds(0, sb), xtb, ytb, wB, "b")
    nc.compile()
    return nc


def _tiles_for(cnt: int) -> tuple:
    """Token tiles covering cnt: full 512s plus one final tile (fp16 runs
    full-rate at any N; keep a multiple of 8 for DMA alignment)."""
    full, rem = divmod(cnt, TT)
    if rem == 0:
        return (TT,) * max(full, 1)
    rem = max(64, ((rem + 7) // 8) * 8)
    if rem == TT:
        return (TT,) * (full + 1)
    return (TT,) * full + (rem,)


def _get_program(tiles_a: tuple, sb: int):
    key = (tiles_a, sb)
    if key not in _program_cache:
        _program_cache[key] = _build_program(tiles_a, sb)
    return _program_cache[key]


def _chunk_w(wt: np.ndarray) -> np.ndarray:
    """[K, M] weight (K contracted) -> chunk layout [m_blk, k_in, k_blk, m_in],
    contiguous per m_blk."""
    K, M = wt.shape
    return np.ascontiguousarray(wt.reshape(K // P, P, M // P, P).transpose(2, 1, 0, 3))


def _route(flat: np.ndarray, gate_w: np.ndarray):
    """Mirror the reference router bit-for-bit (jax ops, same backend)."""
    try:
        import jax
        import jax.numpy as jnp

        logits = jnp.asarray(flat) @ jnp.asarray(gate_w).T
        scores = jax.nn.sigmoid(logits)
        top_val, top_idx = jax.lax.top_k(scores, TOP_K)
        top_val = top_val / (top_val.sum(-1, keepdims=True) + 1e-9)
        return np.asarray(top_val), np.asarray(top_idx)
    except Exception:
        # numpy fallback: identical selection semantics (stable descending)
        logits = flat @ gate_w.T
        scores = 1.0 / (1.0 + np.exp(-logits))
        order = np.argsort(-scores, axis=-1, kind="stable")
        top_idx = order[:, :TOP_K].astype(np.int32)
        top_val = np.take_along_axis(scores, top_idx, axis=-1)
        top_val = top_val / (top_val.sum(-1, keepdims=True) + 1e-9)
        return top_val.astype(np.float32), top_idx


def kernel(x, gate_w, gate_proj, up_proj, down_proj):
    x = np.asarray(x)
    bsz, seqlen, dim = x.shape
    flat = np.ascontiguousarray(x.reshape(-1, dim), dtype=np.float32)
    T = flat.shape[0]
    gate_w = np.asarray(gate_w, dtype=np.float32)
    gate_proj = np.asarray(gate_proj, dtype=np.float32)
    up_proj = np.asarray(up_proj, dtype=np.float32)
    down_proj = np.asarray(down_proj, dtype=np.float32)

    top_val, top_idx = _route(flat, gate_w)

    idx_list = []
    cw_list = []
    for e in range(NUM_EXPERTS):
        mask = top_idx == e  # [T, K]
        tok = np.nonzero(mask.any(axis=1))[0]
        w = (top_val * mask).sum(axis=1)[tok].astype(np.float32)
        idx_list.append(tok)
        cw_list.append(w)

    counts = [len(i) for i in idx_list]
    # Segment-B assignment: overflow beyond CAP, in chunks of <= sb slots,
    # one chunk per core. Grow sb if the fixed default can't fit.
    sb = SB if max(counts) > CAP else 0
    while sb:
        chunks = []  # (expert, start_in_expert, count)
        for e in range(NUM_EXPERTS):
            ov = counts[e] - CAP
            s = CAP
            while ov > 0:
                c = min(ov, sb)
                chunks.append((e, s, c))
                s += c
                ov -= c
        if len(chunks) <= NUM_EXPERTS:
            break
        sb += 128
    if not sb:
        chunks = []

    cap_eff = CAP if sb else max(counts)
    tiles_a = _tiles_for(cap_eff)
    CA = sum(tiles_a)
    nc = _get_program(tiles_a, sb)

    flat16 = flat.astype(np.float16)
    in_maps = []
    wchunk16 = lambda w: _chunk_w(w.T).astype(np.float16)
    wcache = [
        (wchunk16(gate_proj[e]), wchunk16(up_proj[e]), wchunk16(down_proj[e]))
        for e in range(NUM_EXPERTS)
    ]
    b_assign = []  # per core: (expert, tok_indices) or None
    for c in range(NUM_EXPERTS):
        tok = idx_list[c][:cap_eff]
        cnt = len(tok)
        xt = np.zeros((DIM, CA), dtype=np.float16)
        xt[:, :cnt] = flat16[tok].T
        m = {
            "xt": xt,
            "wg": wcache[c][0],
            "wu": wcache[c][1],
            "wd": wcache[c][2],
        }
        if sb:
            if c < len(chunks):
                e, s, n = chunks[c]
                btok = idx_list[e][s : s + n]
                b_assign.append((e, btok))
                xtb = np.zeros((DIM, sb), dtype=np.float16)
                xtb[:, :n] = flat16[btok].T
                m["xtb"] = xtb
                m["wgb"], m["wub"], m["wdb"] = wcache[e]
            else:
                b_assign.append(None)
                m["xtb"] = np.zeros((DIM, sb), dtype=np.float16)
                m["wgb"], m["wub"], m["wdb"] = wcache[c]
        in_maps.append(m)

    res = run_bass_kernel_spmd(nc, in_maps, core_ids=list(range(NUM_EXPERTS)))
    global LAST_RESULT
    LAST_RESULT = res

    cw_of = {}
    for e in range(NUM_EXPERTS):
        cw_of[e] = dict(zip(idx_list[e].tolist(), cw_list[e].tolist()))
    out = np.zeros((T, DIM), dtype=np.float32)
    for c in range(NUM_EXPERTS):
        tok = idx_list[c][:cap_eff]
        cnt = len(tok)
        if cnt:
            y = res.results[c]["yt"][:, :cnt].astype(np.float32)
            cw = cw_list[c][:cnt]
            out[tok] += (y * cw[None, :]).T
        if sb and b_assign[c] is not None:
            e, btok = b_assign[c]
            n = len(btok)
            y = res.results[c]["ytb"][:, :n].astype(np.float32)
            cw = np.array([cw_of[e][t] for t in btok.tolist()], dtype=np.float32)
            out[btok] += (y * cw[None, :]).T
    return out.reshape(bsz, seqlen, dim)


# revision 6
# speedup vs baseline: 1.0917x; 1.0152x over previous
"""DeepseekV3 mini MoE MLP on 8 TRN2 NeuronCores.

Strategy: expert-parallel with overflow rebalancing. The router (tiny:
0.1% of FLOPs) is computed with jax ops that mirror the reference
bit-for-bit; tokens are then dispatched on the host to per-expert
batches (the "all-to-all"). Core c runs expert c over up to CAP=8192
of its routed tokens (segment A); tokens beyond CAP on heavy experts
are shipped to other cores' fixed-size segment B (SB=128 tokens, with
that expert's weights as per-core data), so every core does the same
8320 token-slots instead of padding all cores to the heaviest expert.
The combine (scatter-add weighted by the top-k routing weights)
happens on the host.

All tensor data is fp16 (10-bit mantissa; rel-err ~5e-4 end to end):
matmuls run at the same 1 elem/cycle TensorE rate as f32r, but
LDWEIGHTS gets FastWeightLoad so the MM stream runs at the
~216ns/512-col floor, and DMA bytes halve. PSUM accumulation is fp32.

Layouts are feature-major ([dim, tokens]) so every matmul contracts
over the SBUF partition dim with no transposes anywhere on device.
x arrives in per-128-row chunk tiles so the first matmul only waits
for 1/8th of the first token tile; weights are passed pre-chunked
([HT, P, DT, P]) so each output-column block's weights arrive in one
fully-contiguous DMA. A short burst of zero-weight warmup matmuls runs
during the initial DMA fill to lift the PE out of its cold (1.2 GHz)
HAM state before real work starts.
"""

import numpy as np

import concourse.bass as bass
import concourse.mybir as mybir
import concourse.tile as tile
from concourse import bacc
from concourse.bass_utils import run_bass_kernel_spmd

DIM = 1024
HIDDEN = 1024
NUM_EXPERTS = 8
TOP_K = 2
P = 128
TT = 512  # main token tile (PSUM bank = 512 fp32)
DT = DIM // P  # 8 d-tiles
HT = HIDDEN // P  # 8 h-tiles
CAP = 8192  # segment-A token slots per core (= avg load, 16 tiles)
SB = 128  # segment-B token slots per core (overflow rebalancing)

F32 = mybir.dt.float32
F16 = mybir.dt.float16

_program_cache: dict[tuple, object] = {}
LAST_RESULT = None


def _build_program(tiles_a: tuple, sb: int):
    """Two-segment FFN: yt = ((silu(x@wg.T) * (x@wu.T)) @ wd.T).T.

    Segment A: sum(tiles_a) tokens with weight set A (the core's own
    expert). Segment B: sb tokens with weight set B (overflow from a
    heavy expert). All params fp16; chunked weight layout
    [m_blk, k_in, k_blk, m_in] so each output block's weights are one
    contiguous DMA.
    """
    CA = sum(tiles_a)
    nc = bacc.Bacc(None, target_bir_lowering=False, debug=False)
    xt = nc.declare_dram_parameter("xt", [DIM, CA], F16, isOutput=False)
    wg = nc.declare_dram_parameter("wg", [HT, P, DT, P], F16, isOutput=False)
    wu = nc.declare_dram_parameter("wu", [HT, P, DT, P], F16, isOutput=False)
    wd = nc.declare_dram_parameter("wd", [HT, P, HT, P], F16, isOutput=False)
    yt = nc.declare_dram_parameter("yt", [DIM, CA], F16, isOutput=True)
    if sb:
        xtb = nc.declare_dram_parameter("xtb", [DIM, sb], F16, isOutput=False)
        wgb = nc.declare_dram_parameter("wgb", [HT, P, DT, P], F16, isOutput=False)
        wub = nc.declare_dram_parameter("wub", [HT, P, DT, P], F16, isOutput=False)
        wdb = nc.declare_dram_parameter("wdb", [HT, P, HT, P], F16, isOutput=False)
        ytb = nc.declare_dram_parameter("ytb", [DIM, sb], F16, isOutput=True)

    with tile.TileContext(nc) as tc:
        with (
            tc.tile_pool(name="wpool", bufs=1) as wpool,
            tc.tile_pool(name="xpool", bufs=2) as xpool,
            tc.tile_pool(name="hpool", bufs=2) as hpool,
            tc.tile_pool(name="apool", bufs=4) as apool,
            tc.tile_pool(name="ypool", bufs=4) as ypool,
            tc.tile_pool(name="pg", bufs=2, space="PSUM") as pgpool,
            tc.tile_pool(name="pu", bufs=2, space="PSUM") as pupool,
            tc.tile_pool(name="py", bufs=3, space="PSUM") as pypool,
        ):
            # PE warmup: zero-weight matmuls issued first in program order.
            # No DMA deps, so they run during the initial x/weight fill and
            # lift the PE HAM out of its cold half-rate state (~3.4us).
            zt = wpool.tile([P, TT], F16, name="zwarm", tag="zwarm")
            nc.vector.memset(zt[:, :], 0.0)
            pwarm = pypool.tile([P, TT], F32, tag="py")
            for _ in range(6):
                nc.tensor.matmul(
                    pwarm[:, :], zt[:, :P], zt[:, :], start=True, stop=True
                )

            def mk_wtiles(pfx):
                wg_c, wu_c, wd_c = [], [], []
                for k in range(HT):
                    wg_c.append(
                        wpool.tile([P, DT * P], F16, name=f"{pfx}wg{k}", tag=f"{pfx}wg{k}")
                    )
                    wu_c.append(
                        wpool.tile([P, DT * P], F16, name=f"{pfx}wu{k}", tag=f"{pfx}wu{k}")
                    )
                for k in range(HT):
                    wd_c.append(
                        wpool.tile([P, HT * P], F16, name=f"{pfx}wd{k}", tag=f"{pfx}wd{k}")
                    )
                return wg_c, wu_c, wd_c

            wA = mk_wtiles("a")
            wB = mk_wtiles("b") if sb else None

            def emit_tile(tt, ts, xsrc, ydst, w3, tagpfx):
                wg_c, wu_c, wd_c = w3
                x_sb = xpool.tile([P, DT * TT], F16, tag="x")
                nc.sync.dma_start(
                    out=x_sb[:, :].rearrange("p (a t) -> p a t", a=DT)[:, :, :tt],
                    in_=xsrc.ap()[:, ts].rearrange("(a p) t -> p a t", p=P),
                )
                h_sb = hpool.tile([P, HT * TT], F16, tag="h")
                for h in range(HT):
                    pg = pgpool.tile([P, tt], F32, tag="pg")
                    pu = pupool.tile([P, tt], F32, tag="pu")
                    for a in range(DT):
                        nc.tensor.matmul(
                            pg[:, :],
                            wg_c[h][:, a * P : (a + 1) * P],
                            x_sb[:, a * TT : a * TT + tt],
                            start=(a == 0),
                            stop=(a == DT - 1),
                        )
                    for a in range(DT):
                        nc.tensor.matmul(
                            pu[:, :],
                            wu_c[h][:, a * P : (a + 1) * P],
                            x_sb[:, a * TT : a * TT + tt],
                            start=(a == 0),
                            stop=(a == DT - 1),
                        )
                    act_sb = apool.tile([P, TT], F32, tag="act")
                    nc.scalar.activation(
                        act_sb[:, :tt], pg[:, :], mybir.ActivationFunctionType.Sigmoid
                    )
                    sil_sb = apool.tile([P, TT], F32, tag="sil")
                    nc.vector.tensor_tensor(
                        sil_sb[:, :tt], act_sb[:, :tt], pg[:, :], mybir.AluOpType.mult
                    )
                    nc.vector.tensor_tensor(
                        h_sb[:, h * TT : h * TT + tt],
                        sil_sb[:, :tt],
                        pu[:, :],
                        mybir.AluOpType.mult,
                    )
                for do in range(HT):
                    py = pypool.tile([P, tt], F32, tag="py")
                    for a in range(HT):
                        nc.tensor.matmul(
                            py[:, :],
                            wd_c[do][:, a * P : (a + 1) * P],
                            h_sb[:, a * TT : a * TT + tt],
                            start=(a == 0),
                            stop=(a == HT - 1),
                        )
                    y_sb = ypool.tile([P, TT], F16, tag="y")
                    nc.scalar.copy(y_sb[:, :tt], py[:, :])
                    nc.sync.dma_start(
                        out=ydst.ap()[do * P : (do + 1) * P, ts], in_=y_sb[:, :tt]
                    )

            first = True
            off = 0
            for t, tt in enumerate(tiles_a):
                ts = bass.ds(off, tt)
                off += tt
                if first:
                    # Weight DMAs (gpsimd queue, off the x-chunk sync queue):
                    # first-block gate/up chunks first so block-0 matmuls can
                    # start as soon as x chunk 0 lands.
                    nc.gpsimd.dma_start(out=wA[0][0][:, :], in_=wg.ap()[0])
                    nc.gpsimd.dma_start(out=wA[1][0][:, :], in_=wu.ap()[0])
                    for k in range(1, HT):
                        nc.gpsimd.dma_start(out=wA[0][k][:, :], in_=wg.ap()[k])
                        nc.gpsimd.dma_start(out=wA[1][k][:, :], in_=wu.ap()[k])
                    for k in range(HT):
                        nc.gpsimd.dma_start(out=wA[2][k][:, :], in_=wd.ap()[k])
                    if sb:
                        for k in range(HT):
                            nc.gpsimd.dma_start(out=wB[0][k][:, :], in_=wgb.ap()[k])
                            nc.gpsimd.dma_start(out=wB[1][k][:, :], in_=wub.ap()[k])
                        for k in range(HT):
                            nc.gpsimd.dma_start(out=wB[2][k][:, :], in_=wdb.ap()[k])
                    first = False
                emit_tile(tt, ts, xt, yt, wA, "a")
            if sb:
                emit_tile(sb, bass.ds(0, sb), xtb, ytb, wB, "b")
    nc.compile()
    return nc


def _tiles_for(cnt: int) -> tuple:
    """Token tiles covering cnt: full 512s plus one final tile (fp16 runs
    full-rate at any N; keep a multiple of 8 for DMA alignment)."""
    full, rem = divmod(cnt, TT)
    if rem == 0:
        return (TT,) * max(full, 1)
    rem = max(64, ((rem + 7) // 8) * 8)
    if rem == TT:
        return (TT,) * (full + 1)
    return (TT,) * full + (rem,)


def _get_program(tiles_a: tuple, sb: int):
    key = (tiles_a, sb)
    if key not in _program_cache:
        _program_cache[key] = _build_program(tiles_a, sb)
    return _program_cache[key]


def _chunk_w(wt: np.ndarray) -> np.ndarray:
    """[K, M] weight (K contracted) -> chunk layout [m_blk, k_in, k_blk, m_in],
    contiguous per m_blk."""
    K, M = wt.shape
    return np.ascontiguousarray(wt.reshape(K // P, P, M // P, P).transpose(2, 1, 0, 3))


def _route(flat: np.ndarray, gate_w: np.ndarray):
    """Mirror the reference router bit-for-bit (jax ops, same backend)."""
    try:
        import jax
        import jax.numpy as jnp

        logits = jnp.asarray(flat) @ jnp.asarray(gate_w).T
        scores = jax.nn.sigmoid(logits)
        top_val, top_idx = jax.lax.top_k(scores, TOP_K)
        top_val = top_val / (top_val.sum(-1, keepdims=True) + 1e-9)
        return np.asarray(top_val), np.asarray(top_idx)
    except Exception:
        # numpy fallback: identical selection semantics (stable descending)
        logits = flat @ gate_w.T
        scores = 1.0 / (1.0 + np.exp(-logits))
        order = np.argsort(-scores, axis=-1, kind="stable")
        top_idx = order[:, :TOP_K].astype(np.int32)
        top_val = np.take_along_axis(scores, top_idx, axis=-1)
        top_val = top_val / (top_val.sum(-1, keepdims=True) + 1e-9)
        return top_val.astype(np.float32), top_idx


def kernel(x, gate_w, gate_proj, up_proj, down_proj):
    x = np.asarray(x)
    bsz, seqlen, dim = x.shape
    flat = np.ascontiguousarray(x.reshape(-1, dim), dtype=np.float32)
    T = flat.shape[0]
    gate_w = np.asarray(gate_w, dtype=np.float32)
    gate_proj = np.asarray(gate_proj, dtype=np.float32)
    up_proj = np.asarray(up_proj, dtype=np.float32)
    down_proj = np.asarray(down_proj, dtype=np.float32)

    top_val, top_idx = _route(flat, gate_w)

    idx_list = []
    cw_list = []
    for e in range(NUM_EXPERTS):
        mask = top_idx == e  # [T, K]
        tok = np.nonzero(mask.any(axis=1))[0]
        w = (top_val * mask).sum(axis=1)[tok].astype(np.float32)
        idx_list.append(tok)
        cw_list.append(w)

    counts = [len(i) for i in idx_list]
    # Segment-B assignment: overflow beyond CAP, in chunks of <= sb slots,
    # one chunk per core. Grow sb if the fixed default can't fit.
    sb = SB if max(counts) > CAP else 0
    while sb:
        chunks = []  # (expert, start_in_expert, count)
        for e in range(NUM_EXPERTS):
            ov = counts[e] - CAP
            s = CAP
            while ov > 0:
                c = min(ov, sb)
                chunks.append((e, s, c))
                s += c
                ov -= c
        if len(chunks) <= NUM_EXPERTS:
            break
        sb += 128
    if not sb:
        chunks = []

    cap_eff = CAP if sb else max(counts)
    tiles_a = _tiles_for(cap_eff)
    CA = sum(tiles_a)
    nc = _get_program(tiles_a, sb)

    flat16 = flat.astype(np.float16)
    in_maps = []
    wchunk16 = lambda w: _chunk_w(w.T).astype(np.float16)
    wcache = [
        (wchunk16(gate_proj[e]), wchunk16(up_proj[e]), wchunk16(down_proj[e]))
        for e in range(NUM_EXPERTS)
    ]
    b_assign = []  # per core: (expert, tok_indices) or None
    for c in range(NUM_EXPERTS):
        tok = idx_list[c][:cap_eff]
        cnt = len(tok)
        xt = np.zeros((DIM, CA), dtype=np.float16)
        xt[:, :cnt] = flat16[tok].T
        m = {
            "xt": xt,
            "wg": wcache[c][0],
            "wu": wcache[c][1],
            "wd": wcache[c][2],
        }
        if sb:
            if c < len(chunks):
                e, s, n = chunks[c]
                btok = idx_list[e][s : s + n]
                b_assign.append((e, btok))
                xtb = np.zeros((DIM, sb), dtype=np.float16)
                xtb[:, :n] = flat16[btok].T
                m["xtb"] = xtb
                m["wgb"], m["wub"], m["wdb"] = wcache[e]
            else:
                b_assign.append(None)
                m["xtb"] = np.zeros((DIM, sb), dtype=np.float16)
                m["wgb"], m["wub"], m["wdb"] = wcache[c]
        in_maps.append(m)

    res = run_bass_kernel_spmd(nc, in_maps, core_ids=list(range(NUM_EXPERTS)))
    global LAST_RESULT
    LAST_RESULT = res

    cw_of = {}
    for e in range(NUM_EXPERTS):
        cw_of[e] = dict(zip(idx_list[e].tolist(), cw_list[e].tolist()))
    out = np.zeros((T, DIM), dtype=np.float32)
    for c in range(NUM_EXPERTS):
        tok = idx_list[c][:cap_eff]
        cnt = len(tok)
        if cnt:
            y = res.results[c]["yt"][:, :cnt].astype(np.float32)
            cw = cw_list[c][:cnt]
            out[tok] += (y * cw[None, :]).T
        if sb and b_assign[c] is not None:
            e, btok = b_assign[c]
            n = len(btok)
            y = res.results[c]["ytb"][:, :n].astype(np.float32)
            cw = np.array([cw_of[e][t] for t in btok.tolist()], dtype=np.float32)
            out[btok] += (y * cw[None, :]).T
    return out.reshape(bsz, seqlen, dim)
